# revision 11
# baseline (speedup 1.0000x reference)
"""Trainium2 Bass kernel for GQA attention block (B=2, S=2048, D=2048,
H=16 q-heads, 4 kv-heads, head_dim=128, rotary, causal).

Sharding: 8 cores = (batch: 2) x (kv-head group: 4). Each core computes its
batch's 4 q-heads (one kv head), plus the output-projection partial product
for its 512 head-dim rows of Wo (Megatron tensor-parallel style). The unshard
sums the 4 partials per batch on the host.

All matmuls run in bf16 with f32 PSUM accumulation. RoPE is applied via a
head-dim permutation folded into Wq/Wk on the host (pure reindexing), so the
rotation becomes elementwise cos/sin multiplies plus a partition half-swap
done with a constant 128x128 permutation matmul. Attention runs in the
[key, query] layout so softmax(QK^T) feeds P@V with no transposes; the
denominator is an elementwise sum of exp tiles plus one ones-matrix matmul.
"""

import sys

try:
    import concourse.bass as bass  # noqa: F401
except ImportError:
    sys.path.insert(0, "/opt/trn_rl_repo")

import numpy as np
import ml_dtypes

import concourse.mybir as mybir
import concourse.tile as tile
from concourse import bacc
from concourse.bass_utils import run_bass_kernel_spmd

F32 = mybir.dt.float32
BF16 = mybir.dt.bfloat16
BF16NP = ml_dtypes.bfloat16

B, S, D = 2, 2048, 2048
H, KVH, HD = 16, 4, 128
G = H // KVH  # q-heads per kv head = 4
THETA = 10000.0
SCALE = 1.0 / np.sqrt(HD)
NCORES = 8
KT = D // 128  # 16 contraction tiles
ST = S // 128  # 16 sequence tiles
QB = S // 512  # 4 query blocks of 512

_CACHED_NC = None


def _build_nc():
    nc = bacc.Bacc("TRN2", target_bir_lowering=False, debug=False,
                   num_devices=NCORES)

    hT = nc.declare_dram_parameter("hT", [D, S], BF16, isOutput=False)
    wq = nc.declare_dram_parameter("wq", [D, G * HD], BF16, isOutput=False)
    wk = nc.declare_dram_parameter("wk", [D, HD], BF16, isOutput=False)
    wv = nc.declare_dram_parameter("wv", [D, HD], BF16, isOutput=False)
    wo = nc.declare_dram_parameter("wo", [G * HD, D], BF16, isOutput=False)
    cosd = nc.declare_dram_parameter("cos", [128, S], BF16, isOutput=False)
    sind = nc.declare_dram_parameter("sin", [128, S], BF16, isOutput=False)
    swapd = nc.declare_dram_parameter("swapm", [128, 128], BF16, isOutput=False)
    identd = nc.declare_dram_parameter("ident", [128, 128], BF16, isOutput=False)
    maskd = nc.declare_dram_parameter("masks", [128, 4 * 512], BF16,
                                      isOutput=False)
    kbiasd = nc.declare_dram_parameter("kbias", [128, ST], F32, isOutput=False)
    outd = nc.declare_dram_parameter("out", [S, D], F32, isOutput=True)

    with tile.TileContext(nc) as tc:
        with (
            tc.tile_pool(name="const", bufs=1) as constp,
            tc.tile_pool(name="qkv", bufs=1) as qkvp,
            tc.tile_pool(name="attn", bufs=1) as attnp,
            tc.tile_pool(name="ht", bufs=1) as htp,
            tc.tile_pool(name="wts", bufs=1) as wtsp,
            tc.tile_pool(name="wo", bufs=1) as wop,
            tc.tile_pool(name="ropet", bufs=4) as ropep,
            tc.tile_pool(name="exps", bufs=6) as expp,
            tc.tile_pool(name="nrm", bufs=3) as nrmp,
            tc.tile_pool(name="oev", bufs=4) as oevp,
            # PSUM: 2 + 1 + 2 + 2 + 1 = 8 banks
            tc.tile_pool(name="psq", bufs=2, space="PSUM") as psq,
            tc.tile_pool(name="psw", bufs=1, space="PSUM") as psw,
            tc.tile_pool(name="pss", bufs=2, space="PSUM") as pss,
            tc.tile_pool(name="psa", bufs=2, space="PSUM") as psa,
            tc.tile_pool(name="psd", bufs=1, space="PSUM") as psd,
        ):
            cos = constp.tile([128, S], BF16, tag="cos")
            sin = constp.tile([128, S], BF16, tag="sin")
            swapm = constp.tile([128, 128], BF16, tag="swapm")
            masks = constp.tile([128, 4 * 512], BF16, tag="masks")
            kbias = constp.tile([128, ST], F32, tag="kbias")
            ones_mat = constp.tile([128, 128], BF16, tag="ones_mat")
            ident = constp.tile([128, 128], BF16, tag="ident")

            # Persistent activations
            qt = [qkvp.tile([128, S], BF16, tag=f"qt{h}", name=f"qt{h}")
                  for h in range(G)]
            kt_t = qkvp.tile([128, S], BF16, tag="kt")
            vt = [qkvp.tile([128, HD], BF16, tag=f"vt{m}", name=f"vt{m}")
                  for m in range(ST)]
            attn_sb = [[attnp.tile([128, 512], BF16, tag=f"at{h}_{p}",
                                   name=f"at{h}_{p}")
                        for p in range(QB)] for h in range(G)]

            # ---------------- inputs ----------------
            # DMA order: smallest/soonest-needed first so early matmuls
            # start while the bulk of hT is still in flight.
            wqs, wks, wvs = [], [], []
            for k in range(KT):
                tk = wtsp.tile([128, HD], BF16, tag=f"wk{k}", name=f"wk{k}")
                nc.sync.dma_start(tk[:], wk[k * 128:(k + 1) * 128, :])
                wks.append(tk)
                tv = wtsp.tile([128, HD], BF16, tag=f"wv{k}", name=f"wv{k}")
                nc.sync.dma_start(tv[:], wv[k * 128:(k + 1) * 128, :])
                wvs.append(tv)
            hts = []
            for k in range(KT):
                t = htp.tile([128, S], BF16, tag=f"ht{k}", name=f"ht{k}")
                nc.sync.dma_start(t[:, 0:1024], hT[k * 128:(k + 1) * 128, 0:1024])
                hts.append(t)
            for k in range(KT):
                nc.sync.dma_start(hts[k][:, 1024:2048],
                                  hT[k * 128:(k + 1) * 128, 1024:2048])
            for k in range(KT):
                tq = wtsp.tile([128, G * HD], BF16, tag=f"wq{k}", name=f"wq{k}")
                nc.sync.dma_start(tq[:], wq[k * 128:(k + 1) * 128, :])
                wqs.append(tq)
            nc.sync.dma_start(swapm[:], swapd[:])
            nc.sync.dma_start(ident[:], identd[:])
            nc.vector.memset(ones_mat[:], 1.0)
            nc.sync.dma_start(cos[:], cosd[:])
            nc.sync.dma_start(sin[:], sind[:])
            nc.sync.dma_start(masks[:], maskd[:])
            nc.sync.dma_start(kbias[:], kbiasd[:])
            wos = []
            for h in range(G):
                t = wop.tile([128, D], BF16, tag=f"wo{h}", name=f"wo{h}")
                nc.sync.dma_start(t[:], wo[h * 128:(h + 1) * 128, :])
                wos.append(t)

            def rope_evict(ps, dst, dst_col0):
                """rope the [128, 512] f32 psum into dst[:, col0:col0+512].
                The single copy releases the PSUM bank; the muls then run
                in the DVE 16-bit fast mode from SBUF."""
                cs = slice(dst_col0, dst_col0 + 512)
                tc_ = ropep.tile([128, 512], BF16, tag="tc", name="tc_")
                nc.vector.tensor_copy(tc_[:], ps[:])
                ta = ropep.tile([128, 512], BF16, tag="ta", name="ta")
                tb = ropep.tile([128, 512], BF16, tag="tb", name="tb")
                nc.vector.tensor_mul(ta[:], tc_[:], cos[:, cs])
                nc.vector.tensor_mul(tb[:], tc_[:], sin[:, cs])
                sw = psw.tile([128, 512], F32, name="sw", tag="psw")
                nc.tensor.matmul(sw[:], swapm[:], tb[:], start=True, stop=True)
                nc.vector.tensor_add(dst[:, cs], ta[:], sw[:])

            # K projection: k-outer over pairs of 512-chunks
            for qc0 in range(0, QB, 2):
                pair = [psq.tile([128, 512], F32, name=f"kp{qc}", tag="psq")
                        for qc in (qc0, qc0 + 1)]
                for k in range(KT):
                    for i, qc in enumerate((qc0, qc0 + 1)):
                        nc.tensor.matmul(
                            pair[i][:], wks[k][:],
                            hts[k][:, qc * 512:(qc + 1) * 512],
                            start=(k == 0), stop=(k == KT - 1),
                        )
                for i, qc in enumerate((qc0, qc0 + 1)):
                    rope_evict(pair[i], kt_t, qc * 512)

            # V projection: compute VT [dv, S] with N=512 matmuls, then
            # transpose 128x128 blocks on the PE back to [s, dv] tiles.
            vtT = qkvp.tile([128, S], BF16, tag="vtT", name="vtT")
            for qc0 in range(0, QB, 2):
                pair = [psq.tile([128, 512], F32, name=f"vp{qc}", tag="psq")
                        for qc in (qc0, qc0 + 1)]
                for k in range(KT):
                    for i, qc in enumerate((qc0, qc0 + 1)):
                        nc.tensor.matmul(
                            pair[i][:], wvs[k][:],
                            hts[k][:, qc * 512:(qc + 1) * 512],
                            start=(k == 0), stop=(k == KT - 1),
                        )
                for i, qc in enumerate((qc0, qc0 + 1)):
                    nc.vector.tensor_copy(vtT[:, qc * 512:(qc + 1) * 512], pair[i][:])
            for m in range(ST):
                tp = psw.tile([128, HD], BF16, name="vtp", tag="psw")
                nc.tensor.transpose(tp[:], vtT[:, m * 128:(m + 1) * 128],
                                    ident[:])
                nc.vector.tensor_copy(vt[m][:], tp[:])

            # Q projection: k-outer per head over pairs of 512-chunks
            for h in range(G):
                for qc0 in range(0, QB, 2):
                    pair = [psq.tile([128, 512], F32, name=f"qp{h}_{qc}", tag="psq")
                            for qc in (qc0, qc0 + 1)]
                    for k in range(KT):
                        for i, qc in enumerate((qc0, qc0 + 1)):
                            nc.tensor.matmul(
                                pair[i][:],
                                wqs[k][:, h * HD:(h + 1) * HD],
                                hts[k][:, qc * 512:(qc + 1) * 512],
                                start=(k == 0), stop=(k == KT - 1),
                            )
                    for i, qc in enumerate((qc0, qc0 + 1)):
                        rope_evict(pair[i], qt[h], qc * 512)

            # ---------------- attention + fused output projection ---------
            for p in range(QB):
                qs = slice(p * 512, (p + 1) * 512)
                n_kt = 4 * (p + 1)
                for h in range(G):
                    a_ps = psa.tile([128, 512], F32, name=f"aps{h}_{p}", tag="psa")
                    exsum_d = nrmp.tile([128, 512], BF16, tag="exsum_d",
                                        name="exsum_d")
                    exsum_g = nrmp.tile([128, 512], BF16, tag="exsum_g",
                                        name="exsum_g")
                    seen_d = seen_g = False
                    for kti in range(n_kt):
                        s_ps = pss.tile([128, 512], F32, name="sps", tag="sps")
                        nc.tensor.matmul(
                            s_ps[:],
                            kt_t[:, kti * 128:(kti + 1) * 128],
                            qt[h][:, qs],
                            start=True, stop=True,
                        )
                        ex = expp.tile([128, 512], BF16, tag="ex", name="ex")
                        nc.scalar.activation(
                            ex[:], s_ps[:],
                            mybir.ActivationFunctionType.Exp,
                            bias=kbias[:, kti:kti + 1], scale=SCALE,
                        )
                        t = kti - 4 * p
                        if t >= 0:
                            nc.gpsimd.tensor_mul(
                                ex[:], ex[:],
                                masks[:, t * 512:(t + 1) * 512])
                        nc.tensor.matmul(
                            a_ps[:], vt[kti][:], ex[:],
                            start=(kti == 0), stop=(kti == n_kt - 1),
                        )
                        # split the running exp-sum between DVE and GpSimd
                        if kti % 2 == 0:
                            if not seen_d:
                                nc.vector.tensor_copy(exsum_d[:], ex[:])
                                seen_d = True
                            else:
                                nc.vector.tensor_add(exsum_d[:], exsum_d[:],
                                                     ex[:])
                        else:
                            if not seen_g:
                                nc.gpsimd.tensor_copy(exsum_g[:], ex[:])
                                seen_g = True
                            else:
                                nc.gpsimd.tensor_add(exsum_g[:], exsum_g[:],
                                                     ex[:])
                    d_ps = psd.tile([128, 512], F32, name="dps", tag="psd")
                    nc.tensor.matmul(d_ps[:], ones_mat[:], exsum_d[:],
                                     start=True, stop=False)
                    nc.tensor.matmul(d_ps[:], ones_mat[:], exsum_g[:],
                                     start=False, stop=True)
                    rec = nrmp.tile([128, 512], F32, tag="rec", name="rec")
                    nc.vector.reciprocal_approx_fast(rec[:], d_ps[:])
                    nc.vector.tensor_mul(attn_sb[h][p][:], a_ps[:], rec[:])
                # output projection for this query block (4 seq tiles)
                for smi in range(4):
                    sm = p * 4 + smi
                    scol = smi * 128
                    for nb in range(4):
                        po = psq.tile([128, 512], F32, name="po", tag="psq")
                        for h in range(G):
                            nc.tensor.matmul(
                                po[:],
                                attn_sb[h][p][:, scol:scol + 128],
                                wos[h][:, nb * 512:(nb + 1) * 512],
                                start=(h == 0), stop=(h == G - 1),
                            )
                        ot = oevp.tile([128, 512], F32, tag="ot", name="ot")
                        nc.vector.tensor_copy(ot[:], po[:])
                        nc.sync.dma_start(
                            outd[sm * 128:(sm + 1) * 128,
                                 nb * 512:(nb + 1) * 512],
                            ot[:],
                        )
    nc.finalize()
    return nc


def _prep_in_maps(hidden_states, attention_mask, position_ids, Wq, Wk, Wv, Wo):
    hidden_states = np.asarray(hidden_states, dtype=np.float32)
    attention_mask = np.asarray(attention_mask)
    position_ids = np.asarray(position_ids)
    Wq = np.asarray(Wq, dtype=np.float32)
    Wk = np.asarray(Wk, dtype=np.float32)
    Wv = np.asarray(Wv, dtype=np.float32)
    Wo = np.asarray(Wo, dtype=np.float32)

    # head-dim permutation: row j<64 <- component 2j, row j>=64 <- 2(j-64)+1
    perm = np.empty(HD, dtype=np.int64)
    perm[:64] = 2 * np.arange(64)
    perm[64:] = 2 * np.arange(64) + 1
    Wq_p = Wq.reshape(D, H, HD)[:, :, perm].reshape(D, H * HD)
    Wk_p = Wk.reshape(D, KVH, HD)[:, :, perm].reshape(D, KVH * HD)

    inv64 = THETA ** (-np.arange(0, HD, 2, dtype=np.float32) / HD)  # [64]
    inv_full = np.concatenate([inv64, inv64])  # [128]

    hT_b, cos_b, sin_b, kb_b = [], [], [], []
    for b in range(B):
        hT_b.append(np.ascontiguousarray(hidden_states[b].T).astype(BF16NP))
        freqs = np.outer(inv_full, position_ids[b].astype(np.float32))
        c = np.cos(freqs)
        s = np.sin(freqs)
        s[64:] = -s[64:]
        c = c.astype(BF16NP)
        s = s.astype(BF16NP)
        cos_b.append(c)
        sin_b.append(s)
        kb = np.where(attention_mask[b] > 0, 0.0, -1e9).astype(np.float32)
        kb_b.append(np.ascontiguousarray(kb.reshape(ST, 128).T))

    swapm = np.zeros((128, 128), dtype=BF16NP)
    idx = np.arange(128)
    swapm[idx, idx ^ 64] = 1

    # causal masks for the 4 straddle positions within a 512-query block:
    # masks[t][k, c] = 1 iff c >= k + 128*t
    masks = np.concatenate(
        [(np.arange(512)[None, :] >= (np.arange(128)[:, None] + 128 * t))
         for t in range(4)], axis=1).astype(BF16NP)

    in_maps = []
    for c in range(NCORES):
        b, g = c // KVH, c % KVH
        in_maps.append({
            "hT": hT_b[b],
            "wq": np.ascontiguousarray(
                Wq_p[:, g * G * HD:(g + 1) * G * HD]).astype(BF16NP),
            "wk": np.ascontiguousarray(
                Wk_p[:, g * HD:(g + 1) * HD]).astype(BF16NP),
            "wv": np.ascontiguousarray(
                Wv[:, g * HD:(g + 1) * HD]).astype(BF16NP),
            "wo": np.ascontiguousarray(
                Wo[g * G * HD:(g + 1) * G * HD, :]).astype(BF16NP),
            "cos": cos_b[b],
            "sin": sin_b[b],
            "swapm": swapm,
            "ident": np.eye(128, dtype=BF16NP),
            "masks": masks,
            "kbias": kb_b[b],
        })
    return in_maps


def _run(inputs, trace=False, tmpdir=None):
    global _CACHED_NC
    if _CACHED_NC is None:
        _CACHED_NC = _build_nc()
    in_maps = _prep_in_maps(
        inputs["hidden_states"], inputs["attention_mask"],
        inputs["position_ids"], inputs["Wq"], inputs["Wk"],
        inputs["Wv"], inputs["Wo"],
    )
    res = run_bass_kernel_spmd(
        _CACHED_NC, in_maps, list(range(NCORES)), trace=trace, tmpdir=tmpdir
    )
    # unshard: per-batch sum of the 4 tensor-parallel partials
    out = np.empty((B, S, D), dtype=np.float32)
    for b in range(B):
        acc = res.results[4 * b]["out"].astype(np.float32)
        for g in range(1, KVH):
            acc = acc + res.results[4 * b + g]["out"]
        out[b] = acc
    return out, res


def kernel(hidden_states, attention_mask, position_ids, segment_ids,
           Wq, Wk, Wv, Wo):
    out, _ = _run({
        "hidden_states": hidden_states,
        "attention_mask": attention_mask,
        "position_ids": position_ids,
        "segment_ids": segment_ids,
        "Wq": Wq, "Wk": Wk, "Wv": Wv, "Wo": Wo,
    })
    return out


# revision 12
# speedup vs baseline: 1.2864x; 1.2864x over previous
"""Trainium2 Bass kernel for GQA attention block (B=2, S=2048, D=2048,
H=16 q-heads, 4 kv-heads, head_dim=128, rotary, causal).

Sharding: 8 cores = (batch: 2) x (kv-head group: 4). Each core computes its
batch's 4 q-heads (one kv head), plus the output-projection partial product
for its 512 head-dim rows of Wo (Megatron tensor-parallel style). The unshard
sums the 4 partials per batch on the host.

All matmuls run in bf16 with f32 PSUM accumulation. RoPE is applied via a
head-dim permutation folded into Wq/Wk on the host (pure reindexing), so the
rotation becomes elementwise cos/sin multiplies plus a partition half-swap
done with a constant 128x128 permutation matmul. Attention runs in the
[key, query] layout so softmax(QK^T) feeds P@V with no transposes; the
denominator is an elementwise sum of exp tiles plus one ones-matrix matmul.
"""

import sys

try:
    import concourse.bass as bass  # noqa: F401
except ImportError:
    sys.path.insert(0, "/opt/trn_rl_repo")

import numpy as np
import ml_dtypes

import concourse.mybir as mybir
import concourse.tile as tile
from concourse import bacc
from concourse.bass_utils import run_bass_kernel_spmd

F32 = mybir.dt.float32
BF16 = mybir.dt.bfloat16
BF16NP = ml_dtypes.bfloat16

B, S, D = 2, 2048, 2048
H, KVH, HD = 16, 4, 128
G = H // KVH  # q-heads per kv head = 4
THETA = 10000.0
SCALE = 1.0 / np.sqrt(HD)
NCORES = 8
KT = D // 128  # 16 contraction tiles
ST = S // 128  # 16 sequence tiles
QB = S // 512  # 4 query blocks of 512

_CACHED_NC = None


def _build_nc():
    nc = bacc.Bacc("TRN2", target_bir_lowering=False, debug=False,
                   num_devices=NCORES)

    hT = nc.declare_dram_parameter("hT", [D, S], BF16, isOutput=False)
    wq = nc.declare_dram_parameter("wq", [D, G * HD], BF16, isOutput=False)
    wk = nc.declare_dram_parameter("wk", [D, HD], BF16, isOutput=False)
    wv = nc.declare_dram_parameter("wv", [D, HD], BF16, isOutput=False)
    wo = nc.declare_dram_parameter("wo", [G * HD, D], BF16, isOutput=False)
    cosd = nc.declare_dram_parameter("cos", [128, S], BF16, isOutput=False)
    sind = nc.declare_dram_parameter("sin", [128, S], BF16, isOutput=False)
    swapd = nc.declare_dram_parameter("swapm", [128, 128], BF16, isOutput=False)
    identd = nc.declare_dram_parameter("ident", [128, 128], BF16, isOutput=False)
    maskd = nc.declare_dram_parameter("masks", [128, 4 * 512], BF16,
                                      isOutput=False)
    kbiasd = nc.declare_dram_parameter("kbias", [128, ST], F32, isOutput=False)
    outd = nc.declare_dram_parameter("out", [S, D], F32, isOutput=True)

    with tile.TileContext(nc) as tc:
        with (
            tc.tile_pool(name="const", bufs=1) as constp,
            tc.tile_pool(name="qkv", bufs=1) as qkvp,
            tc.tile_pool(name="attn", bufs=1) as attnp,
            tc.tile_pool(name="ht", bufs=1) as htp,
            tc.tile_pool(name="wts", bufs=1) as wtsp,
            tc.tile_pool(name="wo", bufs=1) as wop,
            tc.tile_pool(name="ropet", bufs=4) as ropep,
            tc.tile_pool(name="exps", bufs=6) as expp,
            tc.tile_pool(name="nrm", bufs=3) as nrmp,
            tc.tile_pool(name="oev", bufs=4) as oevp,
            # PSUM: 2 + 1 + 2 + 2 + 1 = 8 banks
            tc.tile_pool(name="psq", bufs=2, space="PSUM") as psq,
            tc.tile_pool(name="psw", bufs=1, space="PSUM") as psw,
            tc.tile_pool(name="pss", bufs=2, space="PSUM") as pss,
            tc.tile_pool(name="psa", bufs=2, space="PSUM") as psa,
            tc.tile_pool(name="psd", bufs=1, space="PSUM") as psd,
        ):
            cos = constp.tile([128, S], BF16, tag="cos")
            sin = constp.tile([128, S], BF16, tag="sin")
            swapm = constp.tile([128, 128], BF16, tag="swapm")
            masks = constp.tile([128, 4 * 512], BF16, tag="masks")
            kbias = constp.tile([128, ST], F32, tag="kbias")
            ones_mat = constp.tile([128, 128], BF16, tag="ones_mat")
            ident = constp.tile([128, 128], BF16, tag="ident")

            # Persistent activations
            qt = [qkvp.tile([128, S], BF16, tag=f"qt{h}", name=f"qt{h}")
                  for h in range(G)]
            kt_t = qkvp.tile([128, S], BF16, tag="kt")
            vt = [qkvp.tile([128, HD], BF16, tag=f"vt{m}", name=f"vt{m}")
                  for m in range(ST)]
            attn_sb = [[attnp.tile([128, 512], BF16, tag=f"at{h}_{p}",
                                   name=f"at{h}_{p}")
                        for p in range(QB)] for h in range(G)]

            # ---------------- inputs ----------------
            # DMA order: smallest/soonest-needed first so early matmuls
            # start while the bulk of hT is still in flight.
            wqs, wks, wvs = [], [], []
            for k in range(KT):
                tk = wtsp.tile([128, HD], BF16, tag=f"wk{k}", name=f"wk{k}")
                nc.sync.dma_start(tk[:], wk[k * 128:(k + 1) * 128, :])
                wks.append(tk)
                tv = wtsp.tile([128, HD], BF16, tag=f"wv{k}", name=f"wv{k}")
                nc.sync.dma_start(tv[:], wv[k * 128:(k + 1) * 128, :])
                wvs.append(tv)
            hts = []
            for k in range(KT):
                t = htp.tile([128, S], BF16, tag=f"ht{k}", name=f"ht{k}")
                nc.sync.dma_start(t[:, 0:1024], hT[k * 128:(k + 1) * 128, 0:1024])
                hts.append(t)
            for k in range(KT):
                nc.sync.dma_start(hts[k][:, 1024:2048],
                                  hT[k * 128:(k + 1) * 128, 1024:2048])
            for k in range(KT):
                tq = wtsp.tile([128, G * HD], BF16, tag=f"wq{k}", name=f"wq{k}")
                nc.sync.dma_start(tq[:], wq[k * 128:(k + 1) * 128, :])
                wqs.append(tq)
            nc.sync.dma_start(swapm[:], swapd[:])
            nc.sync.dma_start(ident[:], identd[:])
            nc.vector.memset(ones_mat[:], 1.0)
            nc.sync.dma_start(cos[:], cosd[:])
            nc.sync.dma_start(sin[:], sind[:])
            nc.sync.dma_start(masks[:], maskd[:])
            nc.sync.dma_start(kbias[:], kbiasd[:])
            wos = []
            for h in range(G):
                t = wop.tile([128, D], BF16, tag=f"wo{h}", name=f"wo{h}")
                nc.sync.dma_start(t[:], wo[h * 128:(h + 1) * 128, :])
                wos.append(t)

            def rope_evict(ps, dst, dst_col0):
                """rope the [128, 512] f32 psum into dst[:, col0:col0+512].
                The single copy releases the PSUM bank; the muls then run
                in the DVE 16-bit fast mode from SBUF."""
                cs = slice(dst_col0, dst_col0 + 512)
                tc_ = ropep.tile([128, 512], BF16, tag="tc", name="tc_")
                nc.vector.tensor_copy(tc_[:], ps[:])
                ta = ropep.tile([128, 512], BF16, tag="ta", name="ta")
                tb = ropep.tile([128, 512], BF16, tag="tb", name="tb")
                nc.vector.tensor_mul(ta[:], tc_[:], cos[:, cs])
                nc.vector.tensor_mul(tb[:], tc_[:], sin[:, cs])
                sw = psw.tile([128, 512], F32, name="sw", tag="psw")
                nc.tensor.matmul(sw[:], swapm[:], tb[:], start=True, stop=True)
                nc.vector.tensor_add(dst[:, cs], ta[:], sw[:])

            # K projection: k-outer over pairs of 512-chunks
            for qc0 in range(0, QB, 2):
                pair = [psq.tile([128, 512], F32, name=f"kp{qc}", tag="psq")
                        for qc in (qc0, qc0 + 1)]
                for k in range(KT):
                    for i, qc in enumerate((qc0, qc0 + 1)):
                        nc.tensor.matmul(
                            pair[i][:], wks[k][:],
                            hts[k][:, qc * 512:(qc + 1) * 512],
                            start=(k == 0), stop=(k == KT - 1),
                        )
                for i, qc in enumerate((qc0, qc0 + 1)):
                    rope_evict(pair[i], kt_t, qc * 512)

            # V projection: compute VT [dv, S] with N=512 matmuls, then
            # transpose 128x128 blocks on the PE back to [s, dv] tiles.
            vtT = qkvp.tile([128, S], BF16, tag="vtT", name="vtT")
            for qc0 in range(0, QB, 2):
                pair = [psq.tile([128, 512], F32, name=f"vp{qc}", tag="psq")
                        for qc in (qc0, qc0 + 1)]
                for k in range(KT):
                    for i, qc in enumerate((qc0, qc0 + 1)):
                        nc.tensor.matmul(
                            pair[i][:], wvs[k][:],
                            hts[k][:, qc * 512:(qc + 1) * 512],
                            start=(k == 0), stop=(k == KT - 1),
                        )
                for i, qc in enumerate((qc0, qc0 + 1)):
                    nc.vector.tensor_copy(vtT[:, qc * 512:(qc + 1) * 512], pair[i][:])
            for m in range(ST):
                tp = psw.tile([128, HD], BF16, name="vtp", tag="psw")
                nc.tensor.transpose(tp[:], vtT[:, m * 128:(m + 1) * 128],
                                    ident[:])
                nc.vector.tensor_copy(vt[m][:], tp[:])

            # Q projection: k-outer per head over pairs of 512-chunks
            for h in range(G):
                for qc0 in range(0, QB, 2):
                    pair = [psq.tile([128, 512], F32, name=f"qp{h}_{qc}", tag="psq")
                            for qc in (qc0, qc0 + 1)]
                    for k in range(KT):
                        for i, qc in enumerate((qc0, qc0 + 1)):
                            nc.tensor.matmul(
                                pair[i][:],
                                wqs[k][:, h * HD:(h + 1) * HD],
                                hts[k][:, qc * 512:(qc + 1) * 512],
                                start=(k == 0), stop=(k == KT - 1),
                            )
                    for i, qc in enumerate((qc0, qc0 + 1)):
                        rope_evict(pair[i], qt[h], qc * 512)

            # ---------------- attention + fused output projection ---------
            for p in range(QB):
                qs = slice(p * 512, (p + 1) * 512)
                n_kt = 4 * (p + 1)
                for h in range(G):
                    a_ps = psa.tile([128, 512], F32, name=f"aps{h}_{p}", tag="psa")
                    exsum_d = nrmp.tile([128, 512], BF16, tag="exsum_d",
                                        name="exsum_d")
                    exsum_g = nrmp.tile([128, 512], BF16, tag="exsum_g",
                                        name="exsum_g")
                    seen_d = seen_g = False
                    for kti in range(n_kt):
                        s_ps = pss.tile([128, 512], F32, name="sps", tag="sps")
                        nc.tensor.matmul(
                            s_ps[:],
                            kt_t[:, kti * 128:(kti + 1) * 128],
                            qt[h][:, qs],
                            start=True, stop=True,
                        )
                        ex = expp.tile([128, 512], BF16, tag="ex", name="ex")
                        nc.scalar.activation(
                            ex[:], s_ps[:],
                            mybir.ActivationFunctionType.Exp,
                            bias=kbias[:, kti:kti + 1], scale=SCALE,
                        )
                        t = kti - 4 * p
                        if t >= 0:
                            nc.vector.tensor_mul(
                                ex[:], ex[:],
                                masks[:, t * 512:(t + 1) * 512])
                        nc.tensor.matmul(
                            a_ps[:], vt[kti][:], ex[:],
                            start=(kti == 0), stop=(kti == n_kt - 1),
                        )
                        # two interleaved accumulator chains halve the
                        # serial dependency depth
                        if kti % 2 == 0:
                            if not seen_d:
                                nc.vector.tensor_copy(exsum_d[:], ex[:])
                                seen_d = True
                            else:
                                nc.vector.tensor_add(exsum_d[:], exsum_d[:],
                                                     ex[:])
                        else:
                            if not seen_g:
                                nc.vector.tensor_copy(exsum_g[:], ex[:])
                                seen_g = True
                            else:
                                nc.vector.tensor_add(exsum_g[:], exsum_g[:],
                                                     ex[:])
                    d_ps = psd.tile([128, 512], F32, name="dps", tag="psd")
                    nc.tensor.matmul(d_ps[:], ones_mat[:], exsum_d[:],
                                     start=True, stop=False)
                    nc.tensor.matmul(d_ps[:], ones_mat[:], exsum_g[:],
                                     start=False, stop=True)
                    rec = nrmp.tile([128, 512], F32, tag="rec", name="rec")
                    nc.vector.reciprocal_approx_fast(rec[:], d_ps[:])
                    nc.vector.tensor_mul(attn_sb[h][p][:], a_ps[:], rec[:])
                # output projection for this query block (4 seq tiles)
                for smi in range(4):
                    sm = p * 4 + smi
                    scol = smi * 128
                    for nb in range(4):
                        po = psq.tile([128, 512], F32, name="po", tag="psq")
                        for h in range(G):
                            nc.tensor.matmul(
                                po[:],
                                attn_sb[h][p][:, scol:scol + 128],
                                wos[h][:, nb * 512:(nb + 1) * 512],
                                start=(h == 0), stop=(h == G - 1),
                            )
                        ot = oevp.tile([128, 512], F32, tag="ot", name="ot")
                        nc.vector.tensor_copy(ot[:], po[:])
                        nc.sync.dma_start(
                            outd[sm * 128:(sm + 1) * 128,
                                 nb * 512:(nb + 1) * 512],
                            ot[:],
                        )
    nc.finalize()
    return nc


def _prep_in_maps(hidden_states, attention_mask, position_ids, Wq, Wk, Wv, Wo):
    hidden_states = np.asarray(hidden_states, dtype=np.float32)
    attention_mask = np.asarray(attention_mask)
    position_ids = np.asarray(position_ids)
    Wq = np.asarray(Wq, dtype=np.float32)
    Wk = np.asarray(Wk, dtype=np.float32)
    Wv = np.asarray(Wv, dtype=np.float32)
    Wo = np.asarray(Wo, dtype=np.float32)

    # head-dim permutation: row j<64 <- component 2j, row j>=64 <- 2(j-64)+1
    perm = np.empty(HD, dtype=np.int64)
    perm[:64] = 2 * np.arange(64)
    perm[64:] = 2 * np.arange(64) + 1
    Wq_p = Wq.reshape(D, H, HD)[:, :, perm].reshape(D, H * HD)
    Wk_p = Wk.reshape(D, KVH, HD)[:, :, perm].reshape(D, KVH * HD)

    inv64 = THETA ** (-np.arange(0, HD, 2, dtype=np.float32) / HD)  # [64]
    inv_full = np.concatenate([inv64, inv64])  # [128]

    hT_b, cos_b, sin_b, kb_b = [], [], [], []
    for b in range(B):
        hT_b.append(np.ascontiguousarray(hidden_states[b].T).astype(BF16NP))
        freqs = np.outer(inv_full, position_ids[b].astype(np.float32))
        c = np.cos(freqs)
        s = np.sin(freqs)
        s[64:] = -s[64:]
        c = c.astype(BF16NP)
        s = s.astype(BF16NP)
        cos_b.append(c)
        sin_b.append(s)
        kb = np.where(attention_mask[b] > 0, 0.0, -1e9).astype(np.float32)
        kb_b.append(np.ascontiguousarray(kb.reshape(ST, 128).T))

    swapm = np.zeros((128, 128), dtype=BF16NP)
    idx = np.arange(128)
    swapm[idx, idx ^ 64] = 1

    # causal masks for the 4 straddle positions within a 512-query block:
    # masks[t][k, c] = 1 iff c >= k + 128*t
    masks = np.concatenate(
        [(np.arange(512)[None, :] >= (np.arange(128)[:, None] + 128 * t))
         for t in range(4)], axis=1).astype(BF16NP)

    in_maps = []
    for c in range(NCORES):
        b, g = c // KVH, c % KVH
        in_maps.append({
            "hT": hT_b[b],
            "wq": np.ascontiguousarray(
                Wq_p[:, g * G * HD:(g + 1) * G * HD]).astype(BF16NP),
            "wk": np.ascontiguousarray(
                Wk_p[:, g * HD:(g + 1) * HD]).astype(BF16NP),
            "wv": np.ascontiguousarray(
                Wv[:, g * HD:(g + 1) * HD]).astype(BF16NP),
            "wo": np.ascontiguousarray(
                Wo[g * G * HD:(g + 1) * G * HD, :]).astype(BF16NP),
            "cos": cos_b[b],
            "sin": sin_b[b],
            "swapm": swapm,
            "ident": np.eye(128, dtype=BF16NP),
            "masks": masks,
            "kbias": kb_b[b],
        })
    return in_maps


def _run(inputs, trace=False, tmpdir=None):
    global _CACHED_NC
    if _CACHED_NC is None:
        _CACHED_NC = _build_nc()
    in_maps = _prep_in_maps(
        inputs["hidden_states"], inputs["attention_mask"],
        inputs["position_ids"], inputs["Wq"], inputs["Wk"],
        inputs["Wv"], inputs["Wo"],
    )
    res = run_bass_kernel_spmd(
        _CACHED_NC, in_maps, list(range(NCORES)), trace=trace, tmpdir=tmpdir
    )
    # unshard: per-batch sum of the 4 tensor-parallel partials
    out = np.empty((B, S, D), dtype=np.float32)
    for b in range(B):
        acc = res.results[4 * b]["out"].astype(np.float32)
        for g in range(1, KVH):
            acc = acc + res.results[4 * b + g]["out"]
        out[b] = acc
    return out, res


def kernel(hidden_states, attention_mask, position_ids, segment_ids,
           Wq, Wk, Wv, Wo):
    out, _ = _run({
        "hidden_states": hidden_states,
        "attention_mask": attention_mask,
        "position_ids": position_ids,
        "segment_ids": segment_ids,
        "Wq": Wq, "Wk": Wk, "Wv": Wv, "Wo": Wo,
    })
    return out


# revision 13
# speedup vs baseline: 1.3560x; 1.0541x over previous
"""Trainium2 Bass kernel for GQA attention block (B=2, S=2048, D=2048,
H=16 q-heads, 4 kv-heads, head_dim=128, rotary, causal).

Sharding: 8 cores = (batch: 2) x (kv-head group: 4). Each core computes its
batch's 4 q-heads (one kv head), plus the output-projection partial product
for its 512 head-dim rows of Wo (Megatron tensor-parallel style). The unshard
sums the 4 partials per batch on the host.

All matmuls run in bf16 with f32 PSUM accumulation. RoPE is applied via a
head-dim permutation folded into Wq/Wk on the host (pure reindexing), so the
rotation becomes elementwise cos/sin multiplies plus a partition half-swap
done with a constant 128x128 permutation matmul. Attention runs in the
[key, query] layout so softmax(QK^T) feeds P@V with no transposes; the
denominator is an elementwise sum of exp tiles plus one ones-matrix matmul.
"""

import sys

try:
    import concourse.bass as bass  # noqa: F401
except ImportError:
    sys.path.insert(0, "/opt/trn_rl_repo")

import numpy as np
import ml_dtypes

import concourse.mybir as mybir
import concourse.tile as tile
from concourse import bacc
from concourse.bass_utils import run_bass_kernel_spmd

F32 = mybir.dt.float32
BF16 = mybir.dt.bfloat16
BF16NP = ml_dtypes.bfloat16

B, S, D = 2, 2048, 2048
H, KVH, HD = 16, 4, 128
G = H // KVH  # q-heads per kv head = 4
THETA = 10000.0
SCALE = 1.0 / np.sqrt(HD)
NCORES = 8
KT = D // 128  # 16 contraction tiles
ST = S // 128  # 16 sequence tiles
QB = S // 512  # 4 query blocks of 512

_CACHED_NC = None


def _build_nc():
    nc = bacc.Bacc("TRN2", target_bir_lowering=False, debug=False,
                   num_devices=NCORES)

    hT = nc.declare_dram_parameter("hT", [D, S], BF16, isOutput=False)
    # wk | wv | wq packed per 128-row block: [D, 128+128+512]
    wkvqd = nc.declare_dram_parameter("wkvq", [D, 768], BF16, isOutput=False)
    wo = nc.declare_dram_parameter("wo", [G * HD, D], BF16, isOutput=False)
    # cos | sin | masks | swapm | ident packed: [128, 2048+2048+2048+128+128]
    cpackd = nc.declare_dram_parameter("cpack", [128, 3 * S + 256], BF16,
                                       isOutput=False)
    kbiasd = nc.declare_dram_parameter("kbias", [128, ST], F32, isOutput=False)
    outd = nc.declare_dram_parameter("out", [S, D], F32, isOutput=True)

    with tile.TileContext(nc) as tc:
        with (
            tc.tile_pool(name="const", bufs=1) as constp,
            tc.tile_pool(name="qkv", bufs=1) as qkvp,
            tc.tile_pool(name="attn", bufs=1) as attnp,
            tc.tile_pool(name="ht", bufs=1) as htp,
            tc.tile_pool(name="wts", bufs=1) as wtsp,
            tc.tile_pool(name="wo", bufs=1) as wop,
            tc.tile_pool(name="ropet", bufs=4) as ropep,
            tc.tile_pool(name="exps", bufs=6) as expp,
            tc.tile_pool(name="nrm", bufs=3) as nrmp,
            tc.tile_pool(name="oev", bufs=4) as oevp,
            # PSUM: 2 + 1 + 2 + 2 + 1 = 8 banks
            tc.tile_pool(name="psq", bufs=2, space="PSUM") as psq,
            tc.tile_pool(name="psw", bufs=1, space="PSUM") as psw,
            tc.tile_pool(name="pss", bufs=2, space="PSUM") as pss,
            tc.tile_pool(name="psa", bufs=2, space="PSUM") as psa,
            tc.tile_pool(name="psd", bufs=1, space="PSUM") as psd,
        ):
            cpack = constp.tile([128, 3 * S + 256], BF16, tag="cpack")
            cos = cpack[:, 0:S]
            sin = cpack[:, S:2 * S]
            masks = cpack[:, 2 * S:3 * S]
            swapm = cpack[:, 3 * S:3 * S + 128]
            ident = cpack[:, 3 * S + 128:3 * S + 256]
            kbias = constp.tile([128, ST], F32, tag="kbias")
            ones_mat = constp.tile([128, 128], BF16, tag="ones_mat")

            # Persistent activations
            qt = [qkvp.tile([128, S], BF16, tag=f"qt{h}", name=f"qt{h}")
                  for h in range(G)]
            kt_t = qkvp.tile([128, S], BF16, tag="kt")
            vt = [qkvp.tile([128, HD], BF16, tag=f"vt{m}", name=f"vt{m}")
                  for m in range(ST)]
            attn_sb = [[attnp.tile([128, 512], BF16, tag=f"at{h}_{p}",
                                   name=f"at{h}_{p}")
                        for p in range(QB)] for h in range(G)]

            # ---------------- inputs ----------------
            # DMA order: first hT column-half + packed weights (what the
            # first matmuls need), then the rest. Few large DMAs keep the
            # sync engine's descriptor-issue time small.
            hts = []
            for k in range(KT):
                t = htp.tile([128, S], BF16, tag=f"ht{k}", name=f"ht{k}")
                nc.sync.dma_start(t[:, 0:1024], hT[k * 128:(k + 1) * 128, 0:1024])
                hts.append(t)
            wkvqs = []
            for k in range(KT):
                t = wtsp.tile([128, 768], BF16, tag=f"wkvq{k}", name=f"wkvq{k}")
                nc.sync.dma_start(t[:], wkvqd[k * 128:(k + 1) * 128, :])
                wkvqs.append(t)
            wks = [t[:, 0:HD] for t in wkvqs]
            wvs = [t[:, HD:2 * HD] for t in wkvqs]
            wqs = [t[:, 2 * HD:768] for t in wkvqs]
            for k in range(KT):
                nc.sync.dma_start(hts[k][:, 1024:2048],
                                  hT[k * 128:(k + 1) * 128, 1024:2048])
            nc.sync.dma_start(cpack[:], cpackd[:])
            nc.sync.dma_start(kbias[:], kbiasd[:])
            nc.vector.memset(ones_mat[:], 1.0)
            wos = []
            for h in range(G):
                t = wop.tile([128, D], BF16, tag=f"wo{h}", name=f"wo{h}")
                nc.sync.dma_start(t[:], wo[h * 128:(h + 1) * 128, :])
                wos.append(t)

            def rope_evict(ps, dst, dst_col0):
                """rope the [128, 512] f32 psum into dst[:, col0:col0+512].
                The single copy releases the PSUM bank; the muls then run
                in the DVE 16-bit fast mode from SBUF."""
                cs = slice(dst_col0, dst_col0 + 512)
                tc_ = ropep.tile([128, 512], BF16, tag="tc", name="tc_")
                nc.vector.tensor_copy(tc_[:], ps[:])
                ta = ropep.tile([128, 512], BF16, tag="ta", name="ta")
                tb = ropep.tile([128, 512], BF16, tag="tb", name="tb")
                nc.vector.tensor_mul(ta[:], tc_[:], cos[:, cs])
                nc.vector.tensor_mul(tb[:], tc_[:], sin[:, cs])
                sw = psw.tile([128, 512], F32, name="sw", tag="psw")
                nc.tensor.matmul(sw[:], swapm[:], tb[:], start=True, stop=True)
                nc.vector.tensor_add(dst[:, cs], ta[:], sw[:])

            # K projection: k-outer over pairs of 512-chunks
            for qc0 in range(0, QB, 2):
                pair = [psq.tile([128, 512], F32, name=f"kp{qc}", tag="psq")
                        for qc in (qc0, qc0 + 1)]
                for k in range(KT):
                    for i, qc in enumerate((qc0, qc0 + 1)):
                        nc.tensor.matmul(
                            pair[i][:], wks[k][:],
                            hts[k][:, qc * 512:(qc + 1) * 512],
                            start=(k == 0), stop=(k == KT - 1),
                        )
                for i, qc in enumerate((qc0, qc0 + 1)):
                    rope_evict(pair[i], kt_t, qc * 512)

            # V projection: compute VT [dv, S] with N=512 matmuls, then
            # transpose 128x128 blocks on the PE back to [s, dv] tiles.
            vtT = qkvp.tile([128, S], BF16, tag="vtT", name="vtT")
            for qc0 in range(0, QB, 2):
                pair = [psq.tile([128, 512], F32, name=f"vp{qc}", tag="psq")
                        for qc in (qc0, qc0 + 1)]
                for k in range(KT):
                    for i, qc in enumerate((qc0, qc0 + 1)):
                        nc.tensor.matmul(
                            pair[i][:], wvs[k][:],
                            hts[k][:, qc * 512:(qc + 1) * 512],
                            start=(k == 0), stop=(k == KT - 1),
                        )
                for i, qc in enumerate((qc0, qc0 + 1)):
                    nc.vector.tensor_copy(vtT[:, qc * 512:(qc + 1) * 512], pair[i][:])
            for m in range(ST):
                tp = psw.tile([128, HD], BF16, name="vtp", tag="psw")
                nc.tensor.transpose(tp[:], vtT[:, m * 128:(m + 1) * 128],
                                    ident[:])
                nc.vector.tensor_copy(vt[m][:], tp[:])

            # Q projection: k-outer per head over pairs of 512-chunks
            for h in range(G):
                for qc0 in range(0, QB, 2):
                    pair = [psq.tile([128, 512], F32, name=f"qp{h}_{qc}", tag="psq")
                            for qc in (qc0, qc0 + 1)]
                    for k in range(KT):
                        for i, qc in enumerate((qc0, qc0 + 1)):
                            nc.tensor.matmul(
                                pair[i][:],
                                wqs[k][:, h * HD:(h + 1) * HD],
                                hts[k][:, qc * 512:(qc + 1) * 512],
                                start=(k == 0), stop=(k == KT - 1),
                            )
                    for i, qc in enumerate((qc0, qc0 + 1)):
                        rope_evict(pair[i], qt[h], qc * 512)

            # ---------------- attention + fused output projection ---------
            for p in range(QB):
                qs = slice(p * 512, (p + 1) * 512)
                n_kt = 4 * (p + 1)
                for h in range(G):
                    a_ps = psa.tile([128, 512], F32, name=f"aps{h}_{p}", tag="psa")
                    exsum_d = nrmp.tile([128, 512], BF16, tag="exsum_d",
                                        name="exsum_d")
                    exsum_g = nrmp.tile([128, 512], BF16, tag="exsum_g",
                                        name="exsum_g")
                    seen_d = seen_g = False
                    for kti in range(n_kt):
                        s_ps = pss.tile([128, 512], F32, name="sps", tag="sps")
                        nc.tensor.matmul(
                            s_ps[:],
                            kt_t[:, kti * 128:(kti + 1) * 128],
                            qt[h][:, qs],
                            start=True, stop=True,
                        )
                        ex = expp.tile([128, 512], BF16, tag="ex", name="ex")
                        nc.scalar.activation(
                            ex[:], s_ps[:],
                            mybir.ActivationFunctionType.Exp,
                            bias=kbias[:, kti:kti + 1], scale=SCALE,
                        )
                        t = kti - 4 * p
                        if t >= 0:
                            nc.vector.tensor_mul(
                                ex[:], ex[:],
                                masks[:, t * 512:(t + 1) * 512])
                        nc.tensor.matmul(
                            a_ps[:], vt[kti][:], ex[:],
                            start=(kti == 0), stop=(kti == n_kt - 1),
                        )
                        # two interleaved accumulator chains halve the
                        # serial dependency depth
                        if kti % 2 == 0:
                            if not seen_d:
                                nc.vector.tensor_copy(exsum_d[:], ex[:])
                                seen_d = True
                            else:
                                nc.vector.tensor_add(exsum_d[:], exsum_d[:],
                                                     ex[:])
                        else:
                            if not seen_g:
                                nc.vector.tensor_copy(exsum_g[:], ex[:])
                                seen_g = True
                            else:
                                nc.vector.tensor_add(exsum_g[:], exsum_g[:],
                                                     ex[:])
                    d_ps = psd.tile([128, 512], F32, name="dps", tag="psd")
                    nc.tensor.matmul(d_ps[:], ones_mat[:], exsum_d[:],
                                     start=True, stop=False)
                    nc.tensor.matmul(d_ps[:], ones_mat[:], exsum_g[:],
                                     start=False, stop=True)
                    rec = nrmp.tile([128, 512], F32, tag="rec", name="rec")
                    nc.vector.reciprocal_approx_fast(rec[:], d_ps[:])
                    nc.vector.tensor_mul(attn_sb[h][p][:], a_ps[:], rec[:])
                # output projection for this query block (4 seq tiles)
                for smi in range(4):
                    sm = p * 4 + smi
                    scol = smi * 128
                    for nbp in range(2):
                        ot = oevp.tile([128, 1024], F32, tag="ot", name="ot")
                        for half in range(2):
                            nb = nbp * 2 + half
                            po = psq.tile([128, 512], F32, name="po", tag="psq")
                            for h in range(G):
                                nc.tensor.matmul(
                                    po[:],
                                    attn_sb[h][p][:, scol:scol + 128],
                                    wos[h][:, nb * 512:(nb + 1) * 512],
                                    start=(h == 0), stop=(h == G - 1),
                                )
                            nc.vector.tensor_copy(
                                ot[:, half * 512:(half + 1) * 512], po[:])
                        nc.sync.dma_start(
                            outd[sm * 128:(sm + 1) * 128,
                                 nbp * 1024:(nbp + 1) * 1024],
                            ot[:],
                        )
    nc.finalize()
    return nc


def _prep_in_maps(hidden_states, attention_mask, position_ids, Wq, Wk, Wv, Wo):
    hidden_states = np.asarray(hidden_states, dtype=np.float32)
    attention_mask = np.asarray(attention_mask)
    position_ids = np.asarray(position_ids)
    Wq = np.asarray(Wq, dtype=np.float32)
    Wk = np.asarray(Wk, dtype=np.float32)
    Wv = np.asarray(Wv, dtype=np.float32)
    Wo = np.asarray(Wo, dtype=np.float32)

    # head-dim permutation: row j<64 <- component 2j, row j>=64 <- 2(j-64)+1
    perm = np.empty(HD, dtype=np.int64)
    perm[:64] = 2 * np.arange(64)
    perm[64:] = 2 * np.arange(64) + 1
    Wq_p = Wq.reshape(D, H, HD)[:, :, perm].reshape(D, H * HD)
    Wk_p = Wk.reshape(D, KVH, HD)[:, :, perm].reshape(D, KVH * HD)

    inv64 = THETA ** (-np.arange(0, HD, 2, dtype=np.float32) / HD)  # [64]
    inv_full = np.concatenate([inv64, inv64])  # [128]

    hT_b, cos_b, sin_b, kb_b = [], [], [], []
    for b in range(B):
        hT_b.append(np.ascontiguousarray(hidden_states[b].T).astype(BF16NP))
        freqs = np.outer(inv_full, position_ids[b].astype(np.float32))
        c = np.cos(freqs)
        s = np.sin(freqs)
        s[64:] = -s[64:]
        c = c.astype(BF16NP)
        s = s.astype(BF16NP)
        cos_b.append(c)
        sin_b.append(s)
        kb = np.where(attention_mask[b] > 0, 0.0, -1e9).astype(np.float32)
        kb_b.append(np.ascontiguousarray(kb.reshape(ST, 128).T))

    swapm = np.zeros((128, 128), dtype=BF16NP)
    idx = np.arange(128)
    swapm[idx, idx ^ 64] = 1

    # causal masks for the 4 straddle positions within a 512-query block:
    # masks[t][k, c] = 1 iff c >= k + 128*t
    masks = np.concatenate(
        [(np.arange(512)[None, :] >= (np.arange(128)[:, None] + 128 * t))
         for t in range(4)], axis=1).astype(BF16NP)

    in_maps = []
    for c in range(NCORES):
        b, g = c // KVH, c % KVH
        wkvq = np.concatenate([
            Wk_p[:, g * HD:(g + 1) * HD],
            Wv[:, g * HD:(g + 1) * HD],
            Wq_p[:, g * G * HD:(g + 1) * G * HD],
        ], axis=1).astype(BF16NP)
        cpack = np.concatenate([
            cos_b[b], sin_b[b], masks, swapm, np.eye(128, dtype=BF16NP),
        ], axis=1).astype(BF16NP)
        in_maps.append({
            "hT": hT_b[b],
            "wkvq": np.ascontiguousarray(wkvq),
            "wo": np.ascontiguousarray(
                Wo[g * G * HD:(g + 1) * G * HD, :]).astype(BF16NP),
            "cpack": np.ascontiguousarray(cpack),
            "kbias": kb_b[b],
        })
    return in_maps


def _run(inputs, trace=False, tmpdir=None):
    global _CACHED_NC
    if _CACHED_NC is None:
        _CACHED_NC = _build_nc()
    in_maps = _prep_in_maps(
        inputs["hidden_states"], inputs["attention_mask"],
        inputs["position_ids"], inputs["Wq"], inputs["Wk"],
        inputs["Wv"], inputs["Wo"],
    )
    res = run_bass_kernel_spmd(
        _CACHED_NC, in_maps, list(range(NCORES)), trace=trace, tmpdir=tmpdir
    )
    # unshard: per-batch sum of the 4 tensor-parallel partials
    out = np.empty((B, S, D), dtype=np.float32)
    for b in range(B):
        acc = res.results[4 * b]["out"].astype(np.float32)
        for g in range(1, KVH):
            acc = acc + res.results[4 * b + g]["out"]
        out[b] = acc
    return out, res


def kernel(hidden_states, attention_mask, position_ids, segment_ids,
           Wq, Wk, Wv, Wo):
    out, _ = _run({
        "hidden_states": hidden_states,
        "attention_mask": attention_mask,
        "position_ids": position_ids,
        "segment_ids": segment_ids,
        "Wq": Wq, "Wk": Wk, "Wv": Wv, "Wo": Wo,
    })
    return out
